# revision 6
# baseline (speedup 1.0000x reference)
"""Correntropy loss on 8 Trainium2 NeuronCores — centered-fp8, PE+ACT split.

Reference math (all f32):
    t = (target - 0.5) * 2 ; o = (output - 0.5) * 2
    cost = mean(1 - exp(-sigma * (o - t)^2)),  sigma = 1/1000
Since o - t == 2*(output - target) this equals
    mean(1 - exp(-c * w)),  w = (output - target)^2,  c = 4*sigma = 0.004

The kernel is HBM-bandwidth bound (per-core ceiling ~358 GB/s, ~375
measured), so the host ships each tensor at 1 byte/element: fp8-e4m3 of
(x - 0.5).  Centering before quantization is free (the difference o - t
is shift-invariant) and cuts the fp8 round-to-nearest bias on
sum (o-t)^2 from ~4.5e-3 to ~1.1e-3 relative (numpy-simulated
end-to-end vs the f32 reference; tolerance is 2e-2), so no bias
correction, fp16 sample tiles, or duplicated traffic are needed.
Per-core traffic: 16.46 MB (was 22.6 MB in the fp8+fp16 scheme).

Compute is split so no engine gates the stream (PE alone consumes fp8
at only ~256 GB/s, less than DMA delivers):
  * 9 of 16 row-tiles go to TensorE as a self-gram: host packs 63
    chunks of [o(64)|t(64)] per tile and C += blk.T @ blk accumulates;
    diagonal families of C give sum o^2, sum t^2, sum o*t, hence
    sum (o-t)^2, with no DVE/ACT work.  ~36 us of PE time.
  * 7 tiles go to the otherwise-idle DVE+ACT pair: host packs
    [o(4000)|t(4000)] per partition, DVE subtracts (fp8 -> fp16 d),
    ACT squares with a free accumulate into an SBUF column.  ~15 us
    DVE + ~36 us ACT, both hidden under the DMA stream.
Device computes power sums of w; the host evaluates the 1-exp Taylor
series in f64:  sum(1-exp(-c*w)) = c*S1 - c^2/2*S2 + O(c^3*S3).
S2 (an ~8e-4 relative correction) comes from a second ACT Square pass
on 2 of the 7 ACT tiles, scaled by 8; dropped S3 is ~1e-5 relative.

All input pieces ride ONE DMA queue (sync) in an explicit interleave
(ACT tile, then the matching PE tile as two half-pieces) so bandwidth
goes to each consumer just-in-time; the stream ends with pure-PE
pieces whose drain is ~2 us.  A warm-up burst of matmuls on a zeroed
tile latches the HAM clock gate to 2.4 GHz before the first data
arrives.  Outputs: one [128,128] gram (PSUM -> SBUF copy on DVE) and
9 ACT accumulator columns; the host reduces in f64 and applies the
series (the scalar "all-reduce" of the sharding hint, done exactly on
the host).
"""

import numpy as np

import concourse.bacc as bacc
import concourse.mybir as mybir
import concourse.tile as tile
from concourse.bass_utils import run_bass_kernel_spmd

N_CORES = 8
ROWS = 65536
COLS = 1000
ROWS_PER_CORE = ROWS // N_CORES  # 8192
P = 128  # SBUF partitions

Q = 4  # rows folded into the free dim per partition
FREE = Q * COLS  # 4000 elements of one operand per partition per tile
N_TILES = ROWS_PER_CORE // (P * Q)  # 16

CH = 64  # gram chunk width per operand
N_CHUNK = -(-FREE // CH)  # 63 chunks (last one zero-padded)
GFREE = N_CHUNK * CH  # 4032 padded columns per operand
BLK = 2 * CH  # 128-wide [o64|t64] gram block

N_PE_TILES = 9   # row-tiles 0..8: TensorE self-gram path
N_ACT_TILES = 7  # row-tiles 9..15: DVE sub -> ACT Square path
QCOL = FREE // 4  # 1000: quarter-piece width for the last ACT tile
S2_SCALE = float(N_TILES)  # S2 sampled from 1 of 16 tiles
N_S1_COLS = (N_ACT_TILES - 1) + 4  # 6 full tiles + 4 quarter pieces
ACC_COLS = N_S1_COLS + 1  # + 1 S2 col

# DMA stream order: byte-proportional interleave so the PE (which
# consumes fp8 at only ~65% of the DMA delivery rate) never builds a
# backlog: rounds of (ACT tile, PE half, PE half) for the first six
# ACT tiles, then the PE remainder with the LAST ACT tile split into
# four quarter-pieces woven between them, so every consumer's
# post-stream drain is ~2 us.
STREAM = []
for _i in range(6):
    STREAM.append(("act", N_PE_TILES + _i, 0, 0))
    STREAM.append(("pe", _i, 0, 32))
    STREAM.append(("pe", _i, 32, N_CHUNK - 32))
_LAST_A = N_TILES - 1
STREAM += [
    ("pe", 6, 0, 32),
    ("actq", _LAST_A, 0, 0),
    ("pe", 6, 32, N_CHUNK - 32),
    ("actq", _LAST_A, 1, 0),
    ("pe", 7, 0, 32),
    ("actq", _LAST_A, 2, 0),
    ("pe", 7, 32, N_CHUNK - 32),
    ("actq", _LAST_A, 3, 0),
    ("pe", 8, 0, 32),
    ("pe", 8, 32, N_CHUNK - 32),
]

N_MM = N_PE_TILES * N_CHUNK  # 567 gram matmuls

F32 = mybir.dt.float32
F16 = mybir.dt.float16
F8 = mybir.dt.float8e4


def _build():
    nc = bacc.Bacc()
    pe_elems = N_PE_TILES * P * 2 * GFREE
    act_elems = N_ACT_TILES * P * 2 * FREE
    pe_p = nc.declare_dram_parameter("comb_pe", [pe_elems], F8, isOutput=False)
    act_p = nc.declare_dram_parameter("comb_act", [act_elems], F8, isOutput=False)
    acc_p = nc.declare_dram_parameter("partial", [P, ACC_COLS], F32, isOutput=True)
    gram_p = nc.declare_dram_parameter("gram", [BLK, BLK], F32, isOutput=True)

    with tile.TileContext(nc) as tc:
        with (
            tc.tile_pool(name="io", bufs=6) as io_pool,
            tc.tile_pool(name="work", bufs=1) as work_pool,
            tc.tile_pool(name="accp", bufs=1) as acc_pool,
            tc.psum_pool(name="gr", bufs=2) as psum_pool,
        ):
            acc = acc_pool.tile([P, ACC_COLS], F32)
            gram = psum_pool.tile([BLK, BLK], F32, tag="g1")
            gram_sb = acc_pool.tile([BLK, BLK], F32)

            # PE warm-up: back-to-back matmuls on a zeroed tile keep
            # the PE busy past the HAM's 3.4 us activity window,
            # latching the clock gate to 2.4 GHz before data arrives
            # (cold 128-row blocks take 107 ns vs 56 ns warm).
            warm = acc_pool.tile([P, BLK], F8)
            wpsum = psum_pool.tile([BLK, BLK], F32, tag="gw")
            nc.vector.memset(warm[:], 0)
            for wi in range(64):
                nc.tensor.matmul(
                    wpsum[:], warm[:], warm[:],
                    start=(wi == 0), stop=(wi == 63),
                )

            mm = 0
            ofs_pe = ofs_act = 0
            act_j = 0
            for kind, t, c0, nchunk in STREAM:
                if kind == "pe":
                    z = BLK * nchunk
                    ab = io_pool.tile([P, z], F8, tag="pe", bufs=12)
                    nc.sync.dma_start(
                        out=ab[:],
                        in_=pe_p[ofs_pe : ofs_pe + P * z].rearrange(
                            "(p m) -> p m", p=P
                        ),
                    )
                    ofs_pe += P * z
                    for b in range(nchunk):
                        blk = ab[:, b * BLK : (b + 1) * BLK]
                        nc.tensor.matmul(
                            gram[:], blk, blk,
                            start=(mm == 0), stop=(mm == N_MM - 1),
                        )
                        mm += 1
                    continue
                zc = FREE if kind == "act" else QCOL
                ab = io_pool.tile(
                    [P, 2 * zc], F8,
                    tag="act" if kind == "act" else "actq",
                    bufs=4,
                )
                nc.sync.dma_start(
                    out=ab[:],
                    in_=act_p[ofs_act : ofs_act + P * 2 * zc].rearrange(
                        "(p m) -> p m", p=P
                    ),
                )
                ofs_act += P * 2 * zc
                d = work_pool.tile([P, zc], F16, tag="d" + kind, bufs=2)
                nc.vector.tensor_sub(d[:], ab[:, 0:zc], ab[:, zc : 2 * zc])
                w = work_pool.tile([P, zc], F16, tag="w" + kind, bufs=2)
                nc.scalar.activation(
                    w[:], d[:],
                    mybir.ActivationFunctionType.Square,
                    accum_out=acc[:, act_j : act_j + 1],
                )
                if act_j == 0:  # S2 sample: second Square on the first tile
                    w2 = work_pool.tile([P, zc], F16, tag="w2", bufs=1)
                    nc.scalar.activation(
                        w2[:], w[:],
                        mybir.ActivationFunctionType.Square,
                        accum_out=acc[:, N_S1_COLS : N_S1_COLS + 1],
                    )
                act_j += 1

            # gram close: PSUM -> SBUF on the (idle) DVE, then write out.
            nc.vector.tensor_copy(gram_sb[:], gram[:])
            nc.sync.dma_start(out=gram_p[:], in_=gram_sb[:])
            nc.sync.dma_start(out=acc_p[:], in_=acc[:])
    nc.finalize()
    return nc


_NC = None


def _get_nc():
    global _NC
    if _NC is None:
        _NC = _build()
    return _NC


def _pack_gram_cols(o_t, t_t):
    """Whole row-tile -> chunked gram layout [P, N_CHUNK*BLK] fp8."""
    pad = GFREE - FREE
    o_p = np.pad(o_t, ((0, 0), (0, pad))).reshape(P, N_CHUNK, CH)
    t_p = np.pad(t_t, ((0, 0), (0, pad))).reshape(P, N_CHUNK, CH)
    return np.stack([o_p, t_p], axis=2).reshape(P, N_CHUNK * BLK)


def _shard_inputs(output, target):
    import ml_dtypes  # noqa: F401  (float8 numpy dtype support)

    output = np.asarray(output)
    target = np.asarray(target)
    f8np = mybir.dt.np(F8)
    in_maps = []
    for ci in range(N_CORES):
        sl = slice(ci * ROWS_PER_CORE, (ci + 1) * ROWS_PER_CORE)
        # center before fp8 quantization: (o-t) is shift-invariant and
        # fp8(x-0.5) has ~4x less quantization bias than fp8(x)
        o8 = (output[sl].astype(np.float32) - np.float32(0.5)).astype(f8np)
        t8 = (target[sl].astype(np.float32) - np.float32(0.5)).astype(f8np)
        o8 = o8.reshape(N_TILES, P, FREE)
        t8 = t8.reshape(N_TILES, P, FREE)
        pe_blocks = [
            _pack_gram_cols(o8[t], t8[t]).reshape(-1) for t in range(N_PE_TILES)
        ]
        act_blocks = [
            np.concatenate([o8[t], t8[t]], axis=1).reshape(-1)
            for t in range(N_PE_TILES, N_TILES - 1)
        ]
        tq = N_TILES - 1
        for q in range(4):
            cs = slice(q * QCOL, (q + 1) * QCOL)
            act_blocks.append(
                np.concatenate([o8[tq][:, cs], t8[tq][:, cs]], axis=1).reshape(-1)
            )
        in_maps.append(
            {
                "comb_pe": np.concatenate(pe_blocks),
                "comb_act": np.concatenate(act_blocks),
            }
        )
    return in_maps


def run_device(output, target, trace=False):
    in_maps = _shard_inputs(output, target)
    res = run_bass_kernel_spmd(_get_nc(), in_maps, list(range(N_CORES)), trace=trace)
    partials = [
        (res.results[i]["partial"], res.results[i]["gram"]) for i in range(N_CORES)
    ]
    return partials, res


def _gram_s1(g64):
    dg = np.diag(g64)
    return dg[:CH].sum() + dg[CH:].sum() - 2.0 * np.diag(g64[:CH, CH:]).sum()


def _reduce(partials):
    s1 = s2 = 0.0
    for p, g in partials:
        p64 = p.astype(np.float64)
        s1 += _gram_s1(g.astype(np.float64))
        s1 += p64[:, :N_S1_COLS].sum()
        s2 += p64[:, N_S1_COLS:].sum()
    s2 *= S2_SCALE
    c = 4.0 * float(np.float32(1.0 / COLS))  # match reference's f32 sigma
    total = c * s1 - (c * c / 2.0) * s2
    n = float(ROWS) * float(COLS)
    return np.array(total / n, dtype=np.float32)


def kernel(output, target):
    partials, _ = run_device(output, target)
    return _reduce(partials)


# revision 7
# speedup vs baseline: 1.0691x; 1.0691x over previous
"""Correntropy loss on 8 Trainium2 NeuronCores — centered-fp8, PE+ACT split.

Reference math (all f32):
    t = (target - 0.5) * 2 ; o = (output - 0.5) * 2
    cost = mean(1 - exp(-sigma * (o - t)^2)),  sigma = 1/1000
Since o - t == 2*(output - target) this equals
    mean(1 - exp(-c * w)),  w = (output - target)^2,  c = 4*sigma = 0.004

The kernel is HBM-bandwidth bound (per-core ceiling ~358 GB/s, ~375
measured), so the host ships each tensor at 1 byte/element: fp8-e4m3 of
(x - 0.5).  Centering before quantization is free (the difference o - t
is shift-invariant) and cuts the fp8 round-to-nearest bias on
sum (o-t)^2 from ~4.5e-3 to ~1.1e-3 relative (numpy-simulated
end-to-end vs the f32 reference; tolerance is 2e-2), so no bias
correction, fp16 sample tiles, or duplicated traffic are needed.
Per-core traffic: 16.46 MB (was 22.6 MB in the fp8+fp16 scheme).

Compute is split so no engine gates the stream (PE alone consumes fp8
at only ~256 GB/s, less than DMA delivers):
  * 9 of 16 row-tiles go to TensorE as a self-gram: host packs 63
    chunks of [o(64)|t(64)] per tile and C += blk.T @ blk accumulates;
    diagonal families of C give sum o^2, sum t^2, sum o*t, hence
    sum (o-t)^2, with no DVE/ACT work.  ~36 us of PE time.
  * 7 tiles go to the otherwise-idle DVE+ACT pair: host packs
    [o(4000)|t(4000)] per partition, DVE subtracts (fp8 -> fp16 d),
    ACT squares with a free accumulate into an SBUF column.  ~15 us
    DVE + ~36 us ACT, both hidden under the DMA stream.
Device computes power sums of w; the host evaluates the 1-exp Taylor
series in f64:  sum(1-exp(-c*w)) = c*S1 - c^2/2*S2 + O(c^3*S3).
S2 (an ~8e-4 relative correction) comes from a second ACT Square pass
on 2 of the 7 ACT tiles, scaled by 8; dropped S3 is ~1e-5 relative.

All input pieces ride ONE DMA queue (sync) in an explicit interleave
(ACT tile, then the matching PE tile as two half-pieces) so bandwidth
goes to each consumer just-in-time; the stream ends with pure-PE
pieces whose drain is ~2 us.  A warm-up burst of matmuls on a zeroed
tile latches the HAM clock gate to 2.4 GHz before the first data
arrives.  Outputs: one [128,128] gram (PSUM -> SBUF copy on DVE) and
9 ACT accumulator columns; the host reduces in f64 and applies the
series (the scalar "all-reduce" of the sharding hint, done exactly on
the host).
"""

import numpy as np

import concourse.bacc as bacc
import concourse.mybir as mybir
import concourse.tile as tile
from concourse.bass_utils import run_bass_kernel_spmd

N_CORES = 8
ROWS = 65536
COLS = 1000
ROWS_PER_CORE = ROWS // N_CORES  # 8192
P = 128  # SBUF partitions

Q = 4  # rows folded into the free dim per partition
FREE = Q * COLS  # 4000 elements of one operand per partition per tile
N_TILES = ROWS_PER_CORE // (P * Q)  # 16

CH = 64  # gram chunk width per operand
N_CHUNK = -(-FREE // CH)  # 63 chunks (last one zero-padded)
GFREE = N_CHUNK * CH  # 4032 padded columns per operand
BLK = 2 * CH  # 128-wide [o64|t64] gram block

N_PE_TILES = 9   # row-tiles 0..8: TensorE self-gram path
N_ACT_TILES = 7  # row-tiles 9..15: DVE sub -> ACT Square path
QCOL = FREE // 4  # 1000: quarter-piece width for the last ACT tile
S2_SCALE = float(N_TILES)  # S2 sampled from 1 of 16 tiles
N_S1_COLS = (N_ACT_TILES - 1) + 4  # 6 full tiles + 4 quarter pieces
ACC_COLS = N_S1_COLS + 1  # + 1 S2 col

# DMA stream order: byte-proportional interleave so the PE (which
# consumes fp8 at only ~65% of the DMA delivery rate) never builds a
# backlog: rounds of (ACT tile, PE half, PE half) for the first six
# ACT tiles, then the PE remainder with the LAST ACT tile split into
# four quarter-pieces woven between them, so every consumer's
# post-stream drain is ~2 us.
STREAM = []
for _i in range(6):
    STREAM.append(("act", N_PE_TILES + _i, 0, 0))
    STREAM.append(("pe", _i, 0, 32))
    STREAM.append(("pe", _i, 32, N_CHUNK - 32))
_LAST_A = N_TILES - 1
STREAM += [
    ("pe", 6, 0, 32),
    ("actq", _LAST_A, 0, 0),
    ("pe", 6, 32, N_CHUNK - 32),
    ("actq", _LAST_A, 1, 0),
    ("pe", 7, 0, 32),
    ("actq", _LAST_A, 2, 0),
    ("pe", 7, 32, N_CHUNK - 32),
    ("actq", _LAST_A, 3, 0),
    ("pe", 8, 0, 32),
    ("pe", 8, 32, N_CHUNK - 32),
]

N_MM = N_PE_TILES * N_CHUNK  # 567 gram matmuls

F32 = mybir.dt.float32
F16 = mybir.dt.float16
F8 = mybir.dt.float8e4


def _build():
    nc = bacc.Bacc()
    pe_elems = N_PE_TILES * P * 2 * GFREE
    act_elems = N_ACT_TILES * P * 2 * FREE
    pe_p = nc.declare_dram_parameter("comb_pe", [pe_elems], F8, isOutput=False)
    act_p = nc.declare_dram_parameter("comb_act", [act_elems], F8, isOutput=False)
    acc_p = nc.declare_dram_parameter("partial", [P, ACC_COLS], F32, isOutput=True)
    gram_p = nc.declare_dram_parameter("gram", [BLK, BLK], F32, isOutput=True)

    with tile.TileContext(nc) as tc:
        with (
            tc.tile_pool(name="io", bufs=6) as io_pool,
            tc.tile_pool(name="work", bufs=1) as work_pool,
            tc.tile_pool(name="accp", bufs=1) as acc_pool,
            tc.psum_pool(name="gr", bufs=2) as psum_pool,
        ):
            acc = acc_pool.tile([P, ACC_COLS], F32)
            gram = psum_pool.tile([BLK, BLK], F32, tag="g1")
            gram_sb = acc_pool.tile([BLK, BLK], F32)

            # PE warm-up: back-to-back matmuls on a zeroed tile keep
            # the PE busy past the HAM's 3.4 us activity window,
            # latching the clock gate to 2.4 GHz before data arrives
            # (cold 128-row blocks take 107 ns vs 56 ns warm).
            warm = acc_pool.tile([P, BLK], F8)
            wpsum = psum_pool.tile([BLK, BLK], F32, tag="gw")
            nc.vector.memset(warm[:], 0)
            for wi in range(64):
                nc.tensor.matmul(
                    wpsum[:], warm[:], warm[:],
                    start=(wi == 0), stop=(wi == 63),
                )

            mm = 0
            ofs_pe = ofs_act = 0
            act_j = 0
            for kind, t, c0, nchunk in STREAM:
                if kind == "pe":
                    z = BLK * nchunk
                    ab = io_pool.tile([P, z], F8, tag="pe", bufs=12)
                    nc.sync.dma_start(
                        out=ab[:],
                        in_=pe_p[ofs_pe : ofs_pe + P * z].rearrange(
                            "(p m) -> p m", p=P
                        ),
                    )
                    ofs_pe += P * z
                    for b in range(nchunk):
                        blk = ab[:, b * BLK : (b + 1) * BLK]
                        nc.tensor.matmul(
                            gram[:], blk, blk,
                            start=(mm == 0), stop=(mm == N_MM - 1),
                        )
                        mm += 1
                    continue
                zc = FREE if kind == "act" else QCOL
                ab = io_pool.tile(
                    [P, 2 * zc], F8,
                    tag="act" if kind == "act" else "actq",
                    bufs=4,
                )
                nc.sync.dma_start(
                    out=ab[:],
                    in_=act_p[ofs_act : ofs_act + P * 2 * zc].rearrange(
                        "(p m) -> p m", p=P
                    ),
                )
                ofs_act += P * 2 * zc
                # bufs=4: a shallow d pool makes each sub wait on a SQUARE
                # two pieces back, so the sub's DMA-wait executes late, the
                # piece's completion-semaphore lane stays held, and (8 DMA
                # lanes round-robin) input triggers 8 pieces later stall.
                d = work_pool.tile([P, zc], F16, tag="d" + kind, bufs=4)
                nc.vector.tensor_sub(d[:], ab[:, 0:zc], ab[:, zc : 2 * zc])
                w = work_pool.tile([P, zc], F16, tag="w" + kind, bufs=3)
                nc.scalar.activation(
                    w[:], d[:],
                    mybir.ActivationFunctionType.Square,
                    accum_out=acc[:, act_j : act_j + 1],
                )
                if act_j == 0:  # S2 sample: second Square on the first tile
                    w2 = work_pool.tile([P, zc], F16, tag="w2", bufs=1)
                    nc.scalar.activation(
                        w2[:], w[:],
                        mybir.ActivationFunctionType.Square,
                        accum_out=acc[:, N_S1_COLS : N_S1_COLS + 1],
                    )
                act_j += 1

            # gram close: PSUM -> SBUF on the (idle) DVE, then write out.
            nc.vector.tensor_copy(gram_sb[:], gram[:])
            nc.sync.dma_start(out=gram_p[:], in_=gram_sb[:])
            nc.sync.dma_start(out=acc_p[:], in_=acc[:])
    nc.finalize()
    return nc


_NC = None


def _get_nc():
    global _NC
    if _NC is None:
        _NC = _build()
    return _NC


def _pack_gram_cols(o_t, t_t):
    """Whole row-tile -> chunked gram layout [P, N_CHUNK*BLK] fp8."""
    pad = GFREE - FREE
    o_p = np.pad(o_t, ((0, 0), (0, pad))).reshape(P, N_CHUNK, CH)
    t_p = np.pad(t_t, ((0, 0), (0, pad))).reshape(P, N_CHUNK, CH)
    return np.stack([o_p, t_p], axis=2).reshape(P, N_CHUNK * BLK)


def _shard_inputs(output, target):
    import ml_dtypes  # noqa: F401  (float8 numpy dtype support)

    output = np.asarray(output)
    target = np.asarray(target)
    f8np = mybir.dt.np(F8)
    in_maps = []
    for ci in range(N_CORES):
        sl = slice(ci * ROWS_PER_CORE, (ci + 1) * ROWS_PER_CORE)
        # center before fp8 quantization: (o-t) is shift-invariant and
        # fp8(x-0.5) has ~4x less quantization bias than fp8(x)
        o8 = (output[sl].astype(np.float32) - np.float32(0.5)).astype(f8np)
        t8 = (target[sl].astype(np.float32) - np.float32(0.5)).astype(f8np)
        o8 = o8.reshape(N_TILES, P, FREE)
        t8 = t8.reshape(N_TILES, P, FREE)
        pe_blocks = [
            _pack_gram_cols(o8[t], t8[t]).reshape(-1) for t in range(N_PE_TILES)
        ]
        act_blocks = [
            np.concatenate([o8[t], t8[t]], axis=1).reshape(-1)
            for t in range(N_PE_TILES, N_TILES - 1)
        ]
        tq = N_TILES - 1
        for q in range(4):
            cs = slice(q * QCOL, (q + 1) * QCOL)
            act_blocks.append(
                np.concatenate([o8[tq][:, cs], t8[tq][:, cs]], axis=1).reshape(-1)
            )
        in_maps.append(
            {
                "comb_pe": np.concatenate(pe_blocks),
                "comb_act": np.concatenate(act_blocks),
            }
        )
    return in_maps


def run_device(output, target, trace=False):
    in_maps = _shard_inputs(output, target)
    res = run_bass_kernel_spmd(_get_nc(), in_maps, list(range(N_CORES)), trace=trace)
    partials = [
        (res.results[i]["partial"], res.results[i]["gram"]) for i in range(N_CORES)
    ]
    return partials, res


def _gram_s1(g64):
    dg = np.diag(g64)
    return dg[:CH].sum() + dg[CH:].sum() - 2.0 * np.diag(g64[:CH, CH:]).sum()


def _reduce(partials):
    s1 = s2 = 0.0
    for p, g in partials:
        p64 = p.astype(np.float64)
        s1 += _gram_s1(g.astype(np.float64))
        s1 += p64[:, :N_S1_COLS].sum()
        s2 += p64[:, N_S1_COLS:].sum()
    s2 *= S2_SCALE
    c = 4.0 * float(np.float32(1.0 / COLS))  # match reference's f32 sigma
    total = c * s1 - (c * c / 2.0) * s2
    n = float(ROWS) * float(COLS)
    return np.array(total / n, dtype=np.float32)


def kernel(output, target):
    partials, _ = run_device(output, target)
    return _reduce(partials)
